# revision 19
# baseline (speedup 1.0000x reference)
"""Trainium2 Bass kernel for nn_EncoderBlock (pre-norm transformer encoder).

Sharding (8 cores, zero collectives): core c -> batch b = c//4, query-row
block r = (c%4)*1024 .. +1024.  Each core redundantly computes K/V for its
batch over ONLY the keys the attention mask keeps (mask==0 keys are dropped
host-side); pad keys get a -240 score bias folded into the scores matmul
itself (DoubleRow second row), so their attention weight underflows to an
exact fp8 0.

Design notes (cost-model driven):
 - scores run in fp8 DoubleRow at 64-deep contraction: stationary rows are
   (K-tile, padrow), moving rows are (q, indicator).  The indicator row of
   qT is 1 only on partition 0/64, so the pad bias lands once per key.
   This halves PE cost of the scores stream AND makes the exp bias a
   constant, allowing 2-key-tile batched exp instructions.
 - attention runs in QC=4 chunks of 256 queries.  Chunk c's post-process +
   FFN are queued as PE "fillers" inside chunk c+1's ACT-bound exp stream;
   only the last chunk's FFN remains as a serial tail.
 - ACT does (almost) nothing but exp mid-stream; psum->SBUF conversions
   ride on DVE, SBUF adds/copies on Pool (gpsimd); norm2's rsqrt is a
   Newton bit-hack on DVE so no Sqrt table swap evicts the exp table.
 - a tunable fraction of exp groups is offloaded to DVE via a Schraudolph
   bf16 fast-exp (i16 bit trick) + Pool fp8 convert; softmax's num/den
   ratio cancels the systematic approx error (verified < 1e-4 effect).
 - FFN stays bf16: fp8 FFN misses the 2e-2 accuracy budget (measured).
"""

import math
from contextlib import ExitStack

import ml_dtypes
import numpy as np

B, S, D = 2, 4096, 768
H, DK, DFF = 12, 64, 3072
KD = D // 128         # 6 k-tiles over d_model
KD2 = KD // 2         # 3 DoubleRow steps over d_model
FT = DFF // 128       # 24 tiles over d_ff
Q = 1024              # query rows per core
QC = 4                # attention chunks
CQ = Q // QC          # 256 queries per chunk
NCORES = 8
EPS = 1e-6
VAR_SCALE = float(D) / float(D - 1)
BAND = 4
WS = 16.0             # host-side fp8 weight scale (qkv/wo only)
CS = 16.0             # ctx scale in hstage/cT (fp8 range)
PS = 0.25             # P' = PS * softmax numerator (fp8 range, no overflow)
LOG_PS = math.log(PS)
PADB = -240.0         # score bias for pad keys (exp underflows to fp8 0)

# Schraudolph fast-exp (bf16 bit trick): bf16(y_i16) ~= exp(x) for
# y = x*128/ln2 + (127*128 - corr);  kernel input is pss with
# x = pss*0.125 + LOG_PS.
SCHR_SCALE = 0.125 * 128.0 / math.log(2.0)
SCHR_ADD = LOG_PS * 128.0 / math.log(2.0) + 127.0 * 128.0 - 48.0
SCHR_EVERY = 0        # 0 = off; else every Nth exp group goes DVE+Pool
SCHR_MIN_CHUNK = 1    # no offload in chunk 0 (DVE is production-bound)
RSQRT_MAGIC = 0x5F3759DF


def _bands(ntiles, band):
    out = []
    t = 0
    while t < ntiles:
        out.append((t, min(band, ntiles - t)))
        t += band
    return out


def _build(KT, SAFE=None):
    import concourse.bass as bass
    import concourse.mybir as mybir
    import concourse.tile as tile
    from concourse import bacc
    from concourse.bass import ds, ts

    NK = KT * 128
    f32 = mybir.dt.float32
    bf16 = mybir.dt.bfloat16
    fp8 = mybir.dt.float8e4
    i16 = mybir.dt.int16
    i32 = mybir.dt.int32
    AF = mybir.ActivationFunctionType
    OP = mybir.AluOpType
    DR = mybir.MatmulPerfMode.DoubleRow

    nc = bacc.Bacc()

    xq_d = nc.dram_tensor("xq", [Q, D], f32, kind="ExternalInput")  # x + bo
    hkT_d = nc.dram_tensor("hkT", [D, NK], fp8, kind="ExternalInput")
    hqT_d = nc.dram_tensor("hqT", [D, Q], fp8, kind="ExternalInput")
    pad_d = nc.dram_tensor("padrow", [NK], fp8, kind="ExternalInput")
    wq_d = nc.dram_tensor("wqT", [D, D], fp8, kind="ExternalInput")
    wk_d = nc.dram_tensor("wkT", [D, D], fp8, kind="ExternalInput")
    wv_d = nc.dram_tensor("wvT", [D, D], fp8, kind="ExternalInput")
    wo_d = nc.dram_tensor("woT", [D, D], fp8, kind="ExternalInput")
    w1_d = nc.dram_tensor("w1T", [D, DFF], bf16, kind="ExternalInput")
    w2_d = nc.dram_tensor("w2T", [DFF, D], bf16, kind="ExternalInput")
    bq_d = nc.dram_tensor("bq", [D], f32, kind="ExternalInput")
    bk_d = nc.dram_tensor("bk", [D], f32, kind="ExternalInput")
    b1_d = nc.dram_tensor("b1", [DFF], f32, kind="ExternalInput")
    b2_d = nc.dram_tensor("b2", [D], f32, kind="ExternalInput")
    a2_d = nc.dram_tensor("a2", [1], f32, kind="ExternalInput")
    g2_d = nc.dram_tensor("g2", [1], f32, kind="ExternalInput")
    out_d = nc.dram_tensor("out", [Q, D], f32, kind="ExternalOutput")

    def rearr(dslice, n):
        return dslice.rearrange("(t p) d -> p t d", p=128)

    with tile.TileContext(nc) as tc, ExitStack() as ctx:
        const = ctx.enter_context(tc.tile_pool(name="const", bufs=1))

        scal = {}
        for name in ("a2", "g2"):
            scal[name] = const.tile([128, 1], f32, tag=f"sc_{name}",
                                    name=f"sc{name}")

        bqp = const.tile([128, KD], f32, tag="bqp")
        bkp = const.tile([128, KD], f32, tag="bkp")
        b1p = const.tile([128, FT], f32, tag="b1p")

        # warm the exp activation table before anything needs it
        warmt = const.tile([128, 1], f32, tag="warmt")
        expb = const.tile([128, 1], f32, tag="expb")
        nc.vector.memset(warmt, 1.0)
        nc.vector.memset(expb, LOG_PS)
        nc.scalar.activation(warmt, warmt, AF.Exp, bias=expb, scale=0.125)
        ident = const.tile([128, 128], bf16, tag="ident")
        from concourse.masks import make_identity
        make_identity(nc, ident)

        b2b = const.tile([128, D], f32, tag="b2b")
        magicT = const.tile([128, 2], i32, tag="magicT")
        nc.vector.memset(magicT, RSQRT_MAGIC)

        def bcast_dma(dst, d_t, n):
            s = d_t[:]
            nc.sync.dma_start(out=dst, in_=bass.AP(
                tensor=s.tensor, offset=s.offset, ap=[[0, 128], [1, n]]))

        # ---- long-lived activations
        kTp = ctx.enter_context(tc.tile_pool(name="kTp", bufs=1))
        # row KD holds the pad bias row (PADB on pad keys, 0 on kept keys);
        # it rides as the DoubleRow partner of every K tile.
        kT = kTp.tile([128, KD + 1, NK], fp8, tag="kT")
        qTp = ctx.enter_context(tc.tile_pool(name="qTp", bufs=1))
        # row KD is the indicator row: 1 on partitions 0 and 64, else 0.
        qT = qTp.tile([128, KD + 1, Q], fp8, tag="qT")
        nc.vector.memset(qT[:, KD, :], 0.0)
        nc.vector.memset(qT[0:1, KD, :], 1.0)
        nc.vector.memset(qT[64:65, KD, :], 1.0)
        vvp = ctx.enter_context(tc.tile_pool(name="vvp", bufs=1))
        vvo = vvp.tile([128, KT, 12, 68], fp8, tag="vvo")
        wop = ctx.enter_context(tc.tile_pool(name="wop", bufs=1))
        woT_sb = wop.tile([128, KD, D], fp8, tag="woT")
        x1p = ctx.enter_context(tc.tile_pool(name="x1p", bufs=1))
        x1 = x1p.tile([128, Q // 128, D], bf16, tag="x1")
        cTp = ctx.enter_context(tc.tile_pool(name="cTp", bufs=1))
        cT = cTp.tile([128, KD, Q], fp8, tag="cT")
        h2Tp = ctx.enter_context(tc.tile_pool(name="h2Tp", bufs=1))
        h2T = h2Tp.tile([128, KD, Q], bf16, tag="h2T")

        # ones column of V (softmax denominator rides along in the matmul)
        nc.vector.memset(vvo[:, :, :, 64:65], 1.0)

        NG = (KT + 1) // 2  # 2-kt exp groups per (chunk, p)

        with tc.tile_pool(name="ptp", bufs=3) as ptp, \
             tc.tile_pool(name="pt16", bufs=2) as pt16p, \
             tc.tile_pool(name="hsp", bufs=2) as hsp, \
             tc.tile_pool(name="h2sp", bufs=2) as h2sp, \
             tc.tile_pool(name="nrm", bufs=2) as nrmp, \
             tc.tile_pool(name="asm", bufs=8) as asm, \
             tc.tile_pool(name="stats2", bufs=8) as spool2, \
             tc.tile_pool(name="xqb", bufs=2) as xqbp, \
             tc.tile_pool(name="outp", bufs=2) as outp:

            psum_cm = tc.tile_pool(name="psum", bufs=2, space="PSUM")
            psum = psum_cm.__enter__()

            fillers = []

            # Q-side pool closes right after q_proj so w1/w2 can stream in
            # early; K/V-side pool stays open until chunk-1 ends.
            wkv_cm = tc.tile_pool(name="wkv", bufs=1, side="right")
            wkv = wkv_cm.__enter__()
            wqp_cm = tc.tile_pool(name="wqp", bufs=1, side="right")
            wqpool = wqp_cm.__enter__()
            hqT_sb = wqpool.tile([128, KD, Q], fp8, tag="hqT")
            wqs = wqpool.tile([128, KD, D], fp8, tag="wqs")
            hkT_sb = wkv.tile([128, KD, NK], fp8, tag="hkT")
            wks = wkv.tile([128, KD, D], fp8, tag="wks")
            wvs = wkv.tile([128, KD, D], fp8, tag="wvs")

            # DMA order mirrors the dependency chain of the first scores:
            # q-side, then K weights + band-0 keys, then everything else.
            # Multi-tile rearranged transfers keep the HWDGE queue short.
            nc.sync.dma_start(wqs, rearr(wq_d[:], KD))
            nc.sync.dma_start(hqT_sb[:, :, 0:512],
                              hqT_d[:, 0:512].rearrange(
                                  "(t p) q -> p t q", p=128))
            nc.sync.dma_start(bqp, bq_d[:].rearrange("(o p) -> p o", p=128))
            nc.sync.dma_start(wks, rearr(wk_d[:], KD))
            nc.sync.dma_start(hkT_sb[:, :, 0:512],
                              hkT_d[:, 0:512].rearrange(
                                  "(t p) k -> p t k", p=128))
            nc.sync.dma_start(bkp, bk_d[:].rearrange("(o p) -> p o", p=128))
            ps_ = pad_d[:]
            nc.sync.dma_start(kT[:, KD, :], bass.AP(
                tensor=ps_.tensor, offset=ps_.offset, ap=[[0, 128], [1, NK]]))
            nc.sync.dma_start(hqT_sb[:, :, 512:1024],
                              hqT_d[:, 512:1024].rearrange(
                                  "(t p) q -> p t q", p=128))
            nc.sync.dma_start(wvs, rearr(wv_d[:], KD))
            nc.sync.dma_start(hkT_sb[:, :, ds(512, NK - 512)],
                              hkT_d[:, ds(512, NK - 512)].rearrange(
                                  "(t p) k -> p t k", p=128))
            nc.sync.dma_start(woT_sb, rearr(wo_d[:], KD))
            nc.sync.dma_start(b1p, b1_d[:].rearrange("(o p) -> p o", p=128))
            bcast_dma(b2b, b2_d, D)
            bcast_dma(scal["a2"], a2_d, 1)
            bcast_dma(scal["g2"], g2_d, 1)

            def q_proj(b0_, on_act):
                for j in range(KD):
                    pst = psum.tile([128, 512], f32, tag="misc", name="psq")
                    for k in range(KD2):
                        nc.tensor.matmul(pst,
                                         wqs[:, ds(2 * k, 2), ts(j, 128)],
                                         hqT_sb[:, ds(2 * k, 2), ds(b0_, 512)],
                                         start=(k == 0), stop=(k == KD2 - 1),
                                         perf_mode=DR)
                    if on_act and j % 2 == 0:
                        nc.scalar.activation(qT[:, j, ds(b0_, 512)],
                                             pst, AF.Identity,
                                             bias=bqp[:, j:j + 1],
                                             scale=1.0 / WS)
                    else:
                        nc.vector.tensor_scalar(qT[:, j, ds(b0_, 512)],
                                                pst, 1.0 / WS,
                                                bqp[:, j:j + 1],
                                                OP.mult, OP.add)

            def k_band(t0, nt, head):
                def go():
                    w = nt * 128
                    loc0 = t0 * 128
                    for j in range(KD):
                        pst = psum.tile([128, 512], f32, tag="misc",
                                        name="psk")
                        for k in range(KD2):
                            nc.tensor.matmul(pst[:, :w],
                                             wks[:, ds(2 * k, 2), ts(j, 128)],
                                             hkT_sb[:, ds(2 * k, 2),
                                                    ds(loc0, w)],
                                             start=(k == 0),
                                             stop=(k == KD2 - 1),
                                             perf_mode=DR)
                        # split psum->fp8 conversions ACT/DVE: production is
                        # the chunk-0 wall and ACT idles while waiting
                        if j % 2 == 0:
                            nc.scalar.activation(kT[:, j, ds(loc0, w)],
                                                 pst[:, :w], AF.Identity,
                                                 bias=bkp[:, j:j + 1],
                                                 scale=1.0 / WS)
                        else:
                            nc.vector.tensor_scalar(kT[:, j, ds(loc0, w)],
                                                    pst[:, :w], 1.0 / WS,
                                                    bkp[:, j:j + 1],
                                                    OP.mult, OP.add)
                return go

            def v_band(t0, nt):
                def go():
                    for t in range(t0, t0 + nt):
                        loc = t * 128
                        for hh in range(2):
                            pst = psum.tile([128, 512], f32, tag="misc",
                                            name="psv")
                            for k in range(KD2):
                                nc.tensor.matmul(
                                    pst[:, :384],
                                    hkT_sb[:, ds(2 * k, 2), ds(loc, 128)],
                                    wvs[:, ds(2 * k, 2), ts(hh, 384)],
                                    start=(k == 0), stop=(k == KD2 - 1),
                                    perf_mode=DR)
                            # vvo holds 16*v; bv@wo.T is folded into xq on
                            # the host, so this is a pure dtype convert
                            if (t + hh) % 2 == 0:
                                nc.scalar.activation(
                                    vvo[:, t, ds(6 * hh, 6), 0:64],
                                    pst[:, :384].rearrange(
                                        "p (h c) -> p h c", h=6),
                                    AF.Copy, bias=0.0, scale=1.0)
                            else:
                                nc.vector.tensor_copy(
                                    out=vvo[:, t, ds(6 * hh, 6), 0:64],
                                    in_=pst[:, :384].rearrange(
                                        "p (h c) -> p h c", h=6))
                return go

            # band 0 + chunk-0/1 Q inline; the rest rides inside chunk-0's
            # attention stream as PE fillers.  Band j's k_band MUST pop
            # before slot 2j-1 (when its first consumer scores are emitted):
            # [kband1, vband1, kband2, ...] pops kband_j at slot 2(j-1). ok
            kvbands = _bands(KT, BAND)
            q_proj(0, on_act=True)
            k_band(*kvbands[0], head=True)()
            v_band(*kvbands[0])()
            for t0, nt in kvbands[1:]:
                fillers.append(k_band(t0, nt, head=False))
                fillers.append(v_band(t0, nt))
            fillers.append(lambda: q_proj(512, on_act=False))

            # ============ attention chunk machinery ============

            def scores_group(c, p, kt0, nkt, pss):
                # fp8 DR: stationary rows (K-tile, padrow), moving rows
                # (q, indicator) -> pss = K.q + padbias, at half PE cost
                for ik in range(nkt):
                    for hh in range(2):
                        st = kT[ds(64 * hh, 64), p, ts(kt0 + ik, 128)]
                        stb = bass.AP(tensor=st.tensor, offset=st.offset,
                                      ap=[st.ap[0], [(KD - p) * NK, 2],
                                          st.ap[1]])
                        mv = qT[ds(64 * hh, 64), p, ds(c * CQ, CQ)]
                        mvb = bass.AP(tensor=mv.tensor, offset=mv.offset,
                                      ap=[mv.ap[0], [(KD - p) * Q, 2],
                                          mv.ap[1]])
                        nc.tensor.matmul(pss[:, ik, hh, :], stb, mvb,
                                         start=True, stop=True, perf_mode=DR)

            schr_ctr = [0]

            def emit_group(c, p, gi):
                kt0 = 2 * gi
                nkt = min(2, KT - kt0)
                pss = psum.tile([128, 2, 2, CQ], f32, tag="pss")
                scores_group(c, p, kt0, nkt, pss)
                ptt = ptp.tile([128, 2, 2, CQ], fp8, tag="pt")
                schr_ctr[0] += 1
                if (SCHR_EVERY and c >= SCHR_MIN_CHUNK
                        and schr_ctr[0] % SCHR_EVERY == 0):
                    p16 = pt16p.tile([128, 2, 2, CQ], i16, tag="pt16")
                    nc.vector.tensor_scalar(p16[:, 0:nkt], pss[:, 0:nkt],
                                            SCHR_SCALE, SCHR_ADD,
                                            OP.mult, OP.add)
                    nc.gpsimd.tensor_copy(out=ptt[:, 0:nkt],
                                          in_=p16[:, 0:nkt].bitcast(bf16))
                else:
                    nc.scalar.activation(ptt[:, 0:nkt], pss[:, 0:nkt],
                                         AF.Exp, bias=expb, scale=0.125)
                return ptt, kt0, nkt

            def av_group(p, kt0, nkt, ptt, pcc, st_f, sp_f):
                for hh in range(2):
                    head = 2 * p + hh
                    for qb in range(2):
                        if nkt == 2:
                            nc.tensor.matmul(
                                pcc[:, hh, qb, :],
                                ptt[:, :, hh, ds(qb * 128, 128)],
                                vvo[:, ds(kt0, 2), head, 0:65],
                                start=st_f, stop=sp_f, perf_mode=DR)
                        else:
                            nc.tensor.matmul(
                                pcc[:, hh, qb, :],
                                ptt[:, 0, hh, ds(qb * 128, 128)],
                                vvo[:, kt0, head, 0:65],
                                start=st_f, stop=sp_f)

            def divide_out(p, pcc, hstage):
                # hstage = CS * ctx = (CS/WS) * pcc[...,0:64] / pcc[...,64]
                for hh in range(2):
                    for qb in range(2):
                        rr = asm.tile([128, 1], f32, tag="rr")
                        nc.vector.reciprocal(rr, pcc[:, hh, qb, 64:65])
                        nc.vector.tensor_scalar(
                            hstage[:, qb, ds((2 * p + hh) * 64, 64)],
                            pcc[:, hh, qb, 0:64], rr, CS / WS,
                            OP.mult, OP.mult)

            # ---------- post-attention per chunk (fillers) ----------

            def post_a(c, qb, hstage, mvb, tail):
                # ctx transpose -> cT, W_O, x1 = psum + (x+bo), bn_stats
                def go():
                    g = c * 2 + qb
                    xb = xqbp.tile([128, D], f32, tag="xb")
                    nc.sync.dma_start(out=xb, in_=xq_d[ts(g, 128), :])
                    for a in range(2):
                        ptT = psum.tile([128, 3, 128], bf16, tag="misc",
                                        name="ptT")
                        for i in range(3):
                            nc.tensor.transpose(
                                ptT[:, i, :],
                                hstage[:, qb, ts(3 * a + i, 128)], ident)
                        if tail:
                            nc.scalar.activation(
                                cT[:, ds(3 * a, 3), ts(g, 128)], ptT,
                                AF.Copy, bias=0.0, scale=1.0)
                        else:
                            nc.vector.tensor_copy(
                                out=cT[:, ds(3 * a, 3), ts(g, 128)], in_=ptT)
                    for hh in range(2):
                        pst = psum.tile([128, 512], f32, tag="misc",
                                        name="pswo")
                        for k in range(KD2):
                            nc.tensor.matmul(pst[:, :384],
                                             cT[:, ds(2 * k, 2), ts(g, 128)],
                                             woT_sb[:, ds(2 * k, 2),
                                                    ts(hh, 384)],
                                             start=(k == 0),
                                             stop=(k == KD2 - 1),
                                             perf_mode=DR)
                        if tail:
                            nc.scalar.activation(x1[:, g, ts(hh, 384)],
                                                 pst[:, :384], AF.Copy,
                                                 bias=0.0,
                                                 scale=1.0 / (CS * WS))
                        else:
                            nc.vector.tensor_scalar(x1[:, g, ts(hh, 384)],
                                                    pst[:, :384],
                                                    1.0 / (CS * WS),
                                                    None, OP.mult)
                        nc.gpsimd.tensor_tensor(x1[:, g, ts(hh, 384)],
                                                x1[:, g, ts(hh, 384)],
                                                xb[:, ts(hh, 384)], OP.add)
                    st = spool2.tile([128, 3, 6], f32, tag="bnst")
                    for gg in range(3):
                        nc.vector.bn_stats(st[:, gg, :],
                                           x1[:, g, ts(gg, 256)])
                    nc.vector.bn_aggr(mvb[:, qb, :], st)
                return go

            def rsqrt_batch(c, mvb, nrm):
                # rp = alpha2/std, cb = beta2 - mean*rp; Newton rsqrt bit
                # hack on DVE (no ACT Sqrt -> exp table stays resident)
                def go():
                    rp, cb = nrm["rp"], nrm["cb"]
                    vv = h2sp.tile([128, 2], f32, tag="vv", name="vv")
                    vh = h2sp.tile([128, 2], f32, tag="vh", name="vh")
                    t3 = h2sp.tile([128, 2], f32, tag="t3", name="t3")
                    nc.vector.tensor_scalar(vv, mvb[:, :, 1], VAR_SCALE,
                                            None, OP.mult)
                    nc.vector.tensor_scalar(vh, mvb[:, :, 1],
                                            VAR_SCALE * 0.5, None, OP.mult)
                    iv = h2sp.tile([128, 2], i32, tag="iv", name="iv")
                    nc.vector.tensor_scalar(iv, vv.bitcast(i32), 1, None,
                                            OP.arith_shift_right)
                    nc.vector.tensor_tensor(iv, magicT, iv, OP.subtract)
                    y = iv.bitcast(f32)
                    for _ in range(2):
                        nc.vector.tensor_tensor(t3, y, y, OP.mult)
                        nc.vector.tensor_tensor(t3, t3, vh, OP.mult)
                        nc.vector.tensor_scalar(t3, t3, -1.0, 1.5,
                                                OP.mult, OP.add)
                        nc.vector.tensor_tensor(y, y, t3, OP.mult)
                    nc.vector.tensor_scalar(rp, y, scal["a2"], None,
                                            OP.mult)
                    nc.vector.tensor_tensor(cb, mvb[:, :, 0], rp, OP.mult)
                    nc.vector.tensor_scalar(cb, cb, -1.0, scal["g2"],
                                            OP.mult, OP.add)
                return go

            def post_b(c, qb, nrm, tail):
                # h2 = rp*x1 + cb on Pool; transpose -> h2T
                def go():
                    g = c * 2 + qb
                    rp, cb = nrm["rp"], nrm["cb"]
                    h2st = h2sp.tile([128, D], bf16, tag="h2st")
                    nc.gpsimd.tensor_scalar(h2st, x1[:, g, :],
                                            rp[:, qb:qb + 1],
                                            cb[:, qb:qb + 1],
                                            OP.mult, OP.add)
                    for a in range(2):
                        ptT = psum.tile([128, 3, 128], bf16, tag="misc",
                                        name="ptT2")
                        for i in range(3):
                            nc.tensor.transpose(ptT[:, i, :],
                                                h2st[:, ts(3 * a + i, 128)],
                                                ident)
                        if tail:
                            nc.scalar.activation(
                                h2T[:, ds(3 * a, 3), ts(g, 128)], ptT,
                                AF.Copy, bias=0.0, scale=1.0)
                        else:
                            nc.vector.tensor_copy(
                                out=h2T[:, ds(3 * a, 3), ts(g, 128)],
                                in_=ptT)
                return go

            def ff1(c, f, half, h3c, tail):
                # half in (0,1): 128 query cols each
                def go():
                    pst = psum.tile([128, 128], f32, tag="misc", name="psf1")
                    for k in range(KD):
                        nc.tensor.matmul(
                            pst, w1s[:, k, ts(f, 128)],
                            h2T[:, k, ds(c * CQ + half * 128, 128)],
                            start=(k == 0), stop=(k == KD - 1))
                    if tail:
                        nc.scalar.activation(h3c[:, f, ts(half, 128)], pst,
                                             AF.Relu, bias=b1p[:, f:f + 1],
                                             scale=1.0)
                    else:
                        nc.vector.tensor_scalar(h3c[:, f, ts(half, 128)],
                                                pst, b1p[:, f:f + 1], 0.0,
                                                OP.add, OP.max)
                return go

            def ff2_parts(c, qb, hh, h3c, cell, tail):
                g = c * 2 + qb

                def part(k0, k1):
                    def go():
                        if k0 == 0:
                            cell["ps"] = psum.tile([128, 384], f32,
                                                   tag="misc", name="psff2")
                        pst = cell["ps"]
                        for k in range(k0, k1):
                            nc.tensor.matmul(pst,
                                             h3c[:, k, ts(qb, 128)],
                                             w2s[:, k, ts(hh, 384)],
                                             start=(k == 0),
                                             stop=(k == FT - 1))
                        if k1 == FT:
                            if hh == 0:
                                ot = outp.tile([128, D], f32, tag="ot",
                                               name="ot")
                                cell["ot"] = ot
                            else:
                                ot = cell["ot"]
                            nc.vector.tensor_tensor(
                                ot[:, ts(hh, 384)], pst,
                                x1[:, g, ts(hh, 384)], OP.add)
                            nc.gpsimd.tensor_tensor(
                                ot[:, ts(hh, 384)], ot[:, ts(hh, 384)],
                                b2b[:, ts(hh, 384)], OP.add)
                            if hh == 1:
                                nc.sync.dma_start(out_d[ts(g, 128), :], ot)
                    return go
                return [part(k0, min(k0 + 6, FT)) for k0 in range(0, FT, 6)]

            def queue_chunk_post(c, hstage, tail=False):
                mvb = h2sp.tile([128, 2, 2], f32, tag="mvb", name="mvb")
                nrm = {"rp": nrmp.tile([128, 2], f32, tag="rp", name="rp"),
                       "cb": nrmp.tile([128, 2], f32, tag="cb", name="cb")}
                h3c = h3p.tile([128, FT, CQ], bf16, tag="h3")
                fs = [post_a(c, 0, hstage, mvb, tail),
                      post_a(c, 1, hstage, mvb, tail),
                      rsqrt_batch(c, mvb, nrm),
                      post_b(c, 0, nrm, tail),
                      post_b(c, 1, nrm, tail)]
                for f in range(FT):
                    fs.append(ff1(c, f, 0, h3c, tail))
                cell0, cell1 = {}, {}
                # ff2 parts of one (qb, hh) stay adjacent: the held psum
                # tile (tag=misc, bufs=2) must not be rotated out by other
                # misc allocations in between.
                fs.extend(ff2_parts(c, 0, 0, h3c, cell0, tail))
                fs.extend(ff2_parts(c, 0, 1, h3c, cell0, tail))
                for f in range(FT):
                    fs.append(ff1(c, f, 1, h3c, tail))
                fs.extend(ff2_parts(c, 1, 0, h3c, cell1, tail))
                fs.extend(ff2_parts(c, 1, 1, h3c, cell1, tail))
                return fs

            # ============ main attention loop ============
            w1s = w2s = h3p = None
            h3p_cm = w12p_cm = None

            for c in range(QC):
                hstage = hsp.tile([128, 2, D], bf16, tag="hstage")

                pend = None
                for p in range(KD):
                    pcc = psum.tile([128, 2, 2, 65], f32, tag="pc")

                    ptt_next = emit_group(c, p, 0)
                    for i in range(NG):
                        ptt, kt0, nkt = ptt_next
                        if i + 1 < NG:
                            ptt_next = emit_group(c, p, i + 1)
                        if fillers:
                            fillers.pop(0)()
                        av_group(p, kt0, nkt, ptt, pcc,
                                 st_f=(i == 0), sp_f=(i == NG - 1))
                        if pend is not None:
                            divide_out(*pend)
                            pend = None
                    pend = (p, pcc, hstage)
                divide_out(*pend)

                while fillers:
                    fillers.pop(0)()

                if c == 0:
                    # all hqT/hkT consumers (q_proj, k/v bands) are drained;
                    # free both weight pools so w1/w2 can stream in.
                    wqp_cm.__exit__(None, None, None)
                    wkv_cm.__exit__(None, None, None)
                    w12p_cm = tc.tile_pool(name="w12p", bufs=1)
                    w12p = w12p_cm.__enter__()
                    w1s = w12p.tile([128, KD, DFF], bf16, tag="w1s")
                    w2s = w12p.tile([128, FT, D], bf16, tag="w2s")
                    for t0 in range(0, KD, 3):
                        nc.sync.dma_start(
                            w1s[:, ds(t0, 3), :],
                            w1_d[ds(t0 * 128, 384), :].rearrange(
                                "(t p) f -> p t f", p=128))
                    for t0 in range(0, FT, 6):
                        nc.sync.dma_start(
                            w2s[:, ds(t0, 6), :],
                            w2_d[ds(t0 * 128, 768), :].rearrange(
                                "(t p) d -> p t d", p=128))
                    h3p_cm = tc.tile_pool(name="h3p", bufs=2)
                    h3p = h3p_cm.__enter__()

                if c < QC - 1:
                    fillers.extend(queue_chunk_post(c, hstage))
                else:
                    # tail: run the last chunk's post + FFN inline; ACT is
                    # free now, so psum->SBUF copies ride on it.
                    for fn in queue_chunk_post(c, hstage, tail=True):
                        fn()

            psum_cm.__exit__(None, None, None)
            h3p_cm.__exit__(None, None, None)
            w12p_cm.__exit__(None, None, None)

    nc.finalize()
    return nc


def _prep_inputs(inputs):
    bf = ml_dtypes.bfloat16
    f8 = ml_dtypes.float8_e4m3
    x = np.asarray(inputs["x"], np.float32)
    mask = np.asarray(inputs["mask"], np.int32).reshape(B, S)

    kept = [np.nonzero(mask[b])[0] for b in range(B)]
    nk_max = max(len(kept[0]), len(kept[1]))
    KT = max(2, int(math.ceil(nk_max / 128.0)))
    NK = KT * 128

    # norm1 on the host (row-wise affine rescale, like the mask packing)
    a1 = np.asarray(inputs["alpha1"], np.float64).reshape(())
    g1 = np.asarray(inputs["beta1"], np.float64).reshape(())
    x64 = x.astype(np.float64)
    mean = x64.mean(-1, keepdims=True)
    std = x64.std(-1, ddof=1, keepdims=True)
    h = (a1 * (x64 - mean) / (std + EPS) + g1).astype(np.float32)

    # fold bo AND the attention-V bias contribution bv@wo.T into xq:
    # x + (ctx+bv)@wo.T + bo == x + ctx@wo.T + (bo + bv@wo.T)
    bo = (np.asarray(inputs["bo"], np.float64)
          + np.asarray(inputs["bv"], np.float64)
          @ np.asarray(inputs["wo"], np.float64).T).astype(np.float32)

    hkT = []
    pr = []
    for b in range(B):
        n = len(kept[b])
        hkb = np.zeros((NK, D), np.float32)
        hkb[:n] = h[b][kept[b]]
        if n < NK:
            # pad rows get real data; their score bias of -240 makes the
            # attention weight underflow to exactly 0 in fp8
            hkb[n:] = hkb[0]
        hkT.append(np.ascontiguousarray(hkb.T.astype(f8)))
        prb = np.full(NK, PADB, np.float32)
        prb[:n] = 0.0
        pr.append(prb.astype(f8))

    def w_t8(name):
        return np.ascontiguousarray(
            (np.asarray(inputs[name], np.float32).T * WS).astype(f8))

    def w_tb(name):
        return np.ascontiguousarray(
            np.asarray(inputs[name], np.float32).T.astype(bf))

    shared = {
        "wqT": w_t8("wq"), "wkT": w_t8("wk"), "wvT": w_t8("wv"),
        "woT": w_t8("wo"), "w1T": w_tb("w1"), "w2T": w_tb("w2"),
        "bq": np.asarray(inputs["bq"], np.float32),
        "bk": np.asarray(inputs["bk"], np.float32),
        "b1": np.asarray(inputs["b1"], np.float32),
        "b2": np.asarray(inputs["b2"], np.float32),
        "a2": np.asarray(inputs["alpha2"], np.float32).reshape(1),
        "g2": np.asarray(inputs["beta2"], np.float32).reshape(1),
    }

    in_maps = []
    for c in range(NCORES):
        b, r = c // 4, (c % 4) * Q
        m = dict(shared)
        m["xq"] = np.ascontiguousarray(x[b, r:r + Q] + bo[None, :])
        m["hkT"] = hkT[b]
        m["hqT"] = np.ascontiguousarray(h[b, r:r + Q].T.astype(f8))
        m["padrow"] = pr[b]
        in_maps.append(m)
    return KT, None, in_maps


def kernel(**inputs):
    from concourse.bass_utils import run_bass_kernel_spmd

    KT, SAFE, in_maps = _prep_inputs(inputs)
    nc = _build(KT, SAFE)
    res = run_bass_kernel_spmd(nc, in_maps, core_ids=list(range(NCORES)))
    out = np.empty((B, S, D), np.float32)
    for c in range(NCORES):
        b, r = c // 4, (c % 4) * Q
        out[b, r:r + Q] = res.results[c]["out"]
    return out
